# revision 13
# baseline (speedup 1.0000x reference)
# Trainium2 Bass kernel for nn_BoltzmannMachine: sequential Gibbs sweep over
# N=8192 binary units.
#
# Algorithm (exact, matches the jax reference bit-for-bit on binary states):
#   Work in permuted coordinates: unit a is updated at step a.
#   u <= sigmoid(x/T)  <=>  x >= T*logit(u) = thr  (T > 0), so the device
#   only compares against host-precomputed thresholds; no transcendentals.
#   x = x_base + L @ c with c the fire bits and L the strict lower triangle
#   of the permuted coupling matrix (columns scaled by the free mask).
#   Blocked at B=128: PE (TensorE) accumulates each block's x_base row in
#   PSUM out of 128-column matvec contributions (initial-state columns for
#   future blocks, updated columns u = r + f*c for past blocks), with the
#   fp32 weights split into a bf16 hi+lo pair so PE runs at bf16 rate with
#   ~2^-17 relative weight error (x error ~3e-6, far under the minimum
#   compare margin). A sequential DVE sweep resolves each block's 128 bits
#   with ONE fused custom-DVE op per unit: z[j] += L[j,i] * (z[i] >= 0).
#   PE transposes each bit row into a column for downstream block matvecs.
import numpy as np

import concourse.bass as bass  # noqa: F401
import concourse.mybir as mybir
from concourse import bacc, tile
from concourse import bass_utils
from concourse import dve_ops as _dve_ops
from concourse.dve_spec import Spec, Src0, Src1, C0, Zero

F32 = mybir.dt.float32
BF16 = mybir.dt.bfloat16
A = mybir.AluOpType

N_FULL = 8192
B = 128
N_CORES = 8


def _register_gibbs_axpy():
    """Runtime-register the fused sweep op: out = in0 + in1*(s0 >= 0).
    The (C0 + Src1*Zero) form keeps the compare stream-dependent so the
    lowering doesn't hoist it into a latch (IS_GE has no swap complement).
    Src1 (the L row) is always finite, so Src1*Zero == 0 exactly."""
    for op in _dve_ops.OPS:
        if op.name == "GIBBS_AXPY":
            return op
    op = _dve_ops.DveOp(
        "GIBBS_AXPY",
        Spec(
            body=Src0 + Src1 * ((C0 + Src1 * Zero) >= Zero),
            reference=lambda in0, in1, s0, s1, imm2: (
                in0 + in1 * (s0 >= 0.0)
            ).astype(np.float32),
        ),
        subdim=False,
        uops_sha={"v3": "4cebbc5d1fef964b", "v4": "54f17dbd90d668d1"},
    )
    _dve_ops.OPS.append(op)
    _dve_ops.CUSTOM_DVE_SPECS[op.name] = op.spec
    _dve_ops._SUB_OPCODE_FOR_NAME[op.name] = (
        max(_dve_ops._SUB_OPCODE_FOR_NAME.values()) + 1
    )
    return op


GIBBS_AXPY = _register_gibbs_axpy()


def host_prep(w, initial_state, clamping_degree, T, perm, rand_u, N=N_FULL):
    K = N // B
    T = float(np.asarray(T))
    perm = np.asarray(perm).astype(np.int64)

    wp = np.asarray(w, dtype=np.float32)[perm][:, perm]
    s0p = np.asarray(initial_state, dtype=np.float32)[perm]
    f = (np.asarray(clamping_degree)[perm] == 0).astype(np.float32)
    r = s0p * (1.0 - f)
    uu = np.asarray(rand_u, dtype=np.float64)
    with np.errstate(divide="ignore"):
        thr = (T * (np.log(uu) - np.log1p(-uu))).astype(np.float32)

    WPT = np.ascontiguousarray(wp.T)

    # in-block base contributions (upper-incl-diag @ s0p + strict-lower @ r)
    xb = np.zeros(N, dtype=np.float32)
    for k in range(K):
        blk = slice(k * B, (k + 1) * B)
        Wb = wp[blk, blk]
        xb[blk] = (np.triu(Wb, 0) @ s0p[blk] + np.tril(Wb, -1) @ r[blk]).astype(
            np.float32
        )
    bias = (xb - thr).astype(np.float32)

    # Group strips: NG dest-groups of GW=512 columns; strip[g] = WPT[:, g*GW:(g+1)*GW]
    # split into bf16 hi + lo.
    GW = 512
    NG = N // GW
    strips = np.ascontiguousarray(WPT.reshape(N, NG, GW).swapaxes(0, 1))
    whi = strips.astype(mybir.dt.np(BF16))
    wlo = (strips - whi.astype(np.float32)).astype(mybir.dt.np(BF16))

    # Triangular-packed diagonal rows + bias, all on partition 0:
    # ldpack[k] = [bias_row(B) | row0(B-1) | ... | row126(1)] where row i
    # holds L[j,i]*f[i] for j in (i, B)  (fp32 — sweep exactness).
    PACK = B + (B * (B - 1)) // 2
    ldpack = np.zeros((K, PACK), dtype=np.float32)
    for k in range(K):
        blk = slice(k * B, (k + 1) * B)
        ldT = np.triu(WPT[blk, blk] * f[blk][:, None], 1)
        ldpack[k, :B] = bias[k * B:(k + 1) * B]
        off = B
        for i in range(B - 1):
            ldpack[k, off:off + (B - 1 - i)] = ldT[i, i + 1:]
            off += B - 1 - i

    colsT = lambda v: np.ascontiguousarray(v.reshape(K, B).T)

    dev = {
        "whi": whi,
        "wlo": wlo,
        "ldpack": ldpack,
        "s0cols": colsT(s0p).astype(mybir.dt.np(BF16)),  # binary: exact
        "fcols": colsT(f),
        "rcols": colsT(r),
    }
    aux = {"perm": perm, "s0p": s0p, "f": f, "N": N}
    return dev, aux


def assemble_output(c_bits, aux):
    f, s0p, perm, N = aux["f"], aux["s0p"], aux["perm"], aux["N"]
    final_p = f * c_bits.astype(np.float32) + (1.0 - f) * s0p
    out = np.zeros(N, dtype=np.float32)
    out[perm] = final_p
    return out


def build(N=N_FULL):
    K = N // B
    PACK = B + (B * (B - 1)) // 2
    nc = bacc.Bacc("TRN2", target_bir_lowering=False, debug=False)

    GW = 512
    NG = N // GW
    GB = GW // B          # dest blocks per group
    QS = min(8, K)        # source blocks per quarter-tile
    whi_d = nc.dram_tensor("whi", [NG, N, GW], BF16, kind="ExternalInput")
    wlo_d = nc.dram_tensor("wlo", [NG, N, GW], BF16, kind="ExternalInput")
    ldpack_d = nc.dram_tensor("ldpack", [K, PACK], F32, kind="ExternalInput")
    s0cols_d = nc.dram_tensor("s0cols", [B, K], BF16, kind="ExternalInput")
    fcols_d = nc.dram_tensor("fcols", [B, K], F32, kind="ExternalInput")
    rcols_d = nc.dram_tensor("rcols", [B, K], F32, kind="ExternalInput")
    out_d = nc.dram_tensor("c_out", [1, N], F32, kind="ExternalOutput")

    with tile.TileContext(nc) as tc:
        with (
            tc.tile_pool(name="resident", bufs=1) as res,
            tc.tile_pool(name="wpool", bufs=8) as wpool,
            tc.tile_pool(name="ldpool", bufs=2) as ldpool,
            tc.tile_pool(name="zpool", bufs=2) as zpool,
            tc.tile_pool(name="accp", bufs=3, space="PSUM") as accp,
            tc.tile_pool(name="cpsum", bufs=2, space="PSUM") as cpsum,
        ):
            s0_sb = res.tile([B, K], BF16, tag="s0")
            nc.sync.dma_start(out=s0_sb[:, :], in_=s0cols_d.ap())
            f_sb = res.tile([B, K], F32, tag="f")
            nc.sync.dma_start(out=f_sb[:, :], in_=fcols_d.ap())
            r_sb = res.tile([B, K], F32, tag="r")
            nc.sync.dma_start(out=r_sb[:, :], in_=rcols_d.ap())
            u_sb = res.tile([B, K], BF16, tag="u")
            ones_sb = res.tile([1, 1], BF16, tag="ones")
            nc.vector.memset(ones_sb[:, :], 1.0)

            QN = K // QS  # quarters per group strip

            def emit_quarters(g):
                """DMA one group's strip, near quarter (in-group sources) last."""
                tiles = [None] * QN
                gs_ = g * GB
                near = gs_ // QS
                for q in sorted(range(QN), key=lambda q: q == near):
                    qhi = wpool.tile([B, QS * GW], BF16, tag="qhi")
                    nc.sync.dma_start(
                        out=qhi[:, :].rearrange("b (l c) -> b l c", c=GW),
                        in_=whi_d.ap()[g][q * QS * B:(q + 1) * QS * B, :]
                        .rearrange("(l b) c -> b l c", b=B),
                    )
                    qlo = wpool.tile([B, QS * GW], BF16, tag="qlo")
                    nc.sync.dma_start(
                        out=qlo[:, :].rearrange("b (l c) -> b l c", c=GW),
                        in_=wlo_d.ap()[g][q * QS * B:(q + 1) * QS * B, :]
                        .rearrange("(l b) c -> b l c", b=B),
                    )
                    tiles[q] = (qhi, qlo)
                return tiles

            def gacc_matmul(state, l, v):
                qhi, qlo = state["qtiles"][l // QS]
                sl = slice((l % QS) * GW, (l % QS + 1) * GW)
                for half, wt in ((0, qhi), (1, qlo)):
                    nc.tensor.matmul(
                        state["gacc"][:, :], v[:, l:l + 1], wt[:, sl],
                        start=(state["n"] == 0 and half == 0),
                        stop=(l == state["last"] and half == 1),
                    )
                state["n"] += 1

            def new_group_state(g):
                gs_, ge_ = g * GB, (g + 1) * GB
                s0_srcs = [l for l in range(ge_ + GB, K)]
                chunks = [s0_srcs[c::GB] for c in range(GB)]
                # sources ge_..ge_+GB-1 are the NEXT group's in-group units:
                # they go through that group's own saccs, not this gacc.
                # Out-of-group u-side: l < gs_ (old) plus gs_..ge_-1 (current
                # group, emitted as swept). JIT source is ge_-1... for group
                # g the gacc covers sources outside [gs_, ge_): u-side l<gs_,
                # s0-side l >= ge_.
                s0_srcs = [l for l in range(ge_, K)]
                chunks = [s0_srcs[c::GB] for c in range(GB)]
                return {
                    "g": g, "qtiles": emit_quarters(g),
                    "gacc": accp.tile([1, GW], F32, tag="gacc", name="gacc"),
                    "n": 0, "chunks": chunks,
                    "last": gs_ - 1 if gs_ >= 1 else s0_srcs[-1],
                }

            # prologue: group 0 (s0 sources only — no sweep deps)
            cur = new_group_state(0)
            for l in cur["chunks"][0] + cur["chunks"][1] + cur["chunks"][2] + cur["chunks"][3]:
                gacc_matmul(cur, l, s0_sb)
            nxt = None

            for k in range(K):
                g, kg = divmod(k, GB)
                gs, ge = g * GB, (g + 1) * GB

                if kg == 0 and g + 1 < NG:
                    nxt = new_group_state(g + 1)

                ldk = ldpool.tile([1, PACK], F32, tag="ldk")
                qlen = PACK // 4
                for qi in range(4):
                    hi = PACK if qi == 3 else (qi + 1) * qlen
                    nc.sync.dma_start(
                        out=ldk[:, qi * qlen:hi],
                        in_=ldpack_d.ap()[k:k + 1, qi * qlen:hi],
                    )

                # in-group contributions: s0-side first, then u-side, JIT last
                in_order = [l for l in range(k + 1, ge)] + [l for l in range(gs, k)]
                sacc = None
                if in_order:
                    sacc = cpsum.tile([1, B], F32, tag="sacc")
                    for idx, l in enumerate(in_order):
                        v = s0_sb if l > k else u_sb
                        qhi, qlo = cur["qtiles"][l // QS]
                        base = (l % QS) * GW + kg * B
                        for half, wt in ((0, qhi), (1, qlo)):
                            nc.tensor.matmul(
                                sacc[:, :], v[:, l:l + 1], wt[:, base:base + B],
                                start=(idx == 0 and half == 0),
                                stop=(idx == len(in_order) - 1 and half == 1),
                            )

                # seed z = group acc column + bias (+ in-group acc)
                z = zpool.tile([1, B], F32, tag="z")
                nc.vector.tensor_tensor(
                    out=z[:, :], in0=cur["gacc"][0:1, kg * B:(kg + 1) * B],
                    in1=ldk[:, 0:B], op=A.add,
                )
                if sacc is not None:
                    nc.vector.tensor_tensor(
                        out=z[:, :], in0=sacc[0:1, :], in1=z[:, :], op=A.add,
                    )

                # sequential sweep: ONE fused custom op per unit
                for i in range(B - 1):
                    off = B + i * (B - 1) - (i * (i - 1)) // 2
                    nc.vector._custom_dve(
                        GIBBS_AXPY,
                        out=z[:, i + 1:],
                        in0=z[:, i + 1:],
                        in1=ldk[:, off:off + (B - 1 - i)],
                        s0=z[:, i:i + 1],
                    )

                # bits row (bf16 — bits are exact) + u column + output
                cbf = zpool.tile([1, B], BF16, tag="cbf")
                nc.vector.tensor_scalar(
                    out=cbf[:, :], in0=z[:, :],
                    scalar1=0.0, scalar2=None, op0=A.is_ge,
                )
                nc.gpsimd.dma_start(out=out_d.ap()[0:1, k * B:(k + 1) * B], in_=cbf[:, :])
                if k < K - 1:
                    cp = cpsum.tile([B, 1], F32, tag="cp")
                    nc.tensor.matmul(
                        cp[:, :], cbf[:, :], ones_sb[:, :],
                        start=True, stop=True,
                    )
                    nc.vector.scalar_tensor_tensor(
                        out=u_sb[:, k:k + 1], in0=cp[:, :], scalar=f_sb[:, k:k + 1],
                        in1=r_sb[:, k:k + 1], op0=A.mult, op1=A.add,
                    )
                    # feed the freshly swept column into the next group's acc
                    if nxt is not None:
                        gacc_matmul(nxt, k, u_sb)

                # next group's bulk accumulation, behind this block's JIT ops
                if nxt is not None:
                    if kg == 0:
                        for l in range(0, gs):
                            gacc_matmul(nxt, l, u_sb)
                    for l in nxt["chunks"][kg]:
                        gacc_matmul(nxt, l, s0_sb)

                if kg == GB - 1 and nxt is not None:
                    cur = nxt
                    nxt = None

    nc.compile()
    return nc


_NC_CACHE = {}


def _get_nc(N=N_FULL):
    if N not in _NC_CACHE:
        _NC_CACHE[N] = build(N)
    return _NC_CACHE[N]


def kernel(w, initial_state, clamping_degree, T, perm, rand_u, _trace=False):
    dev, aux = host_prep(w, initial_state, clamping_degree, T, perm, rand_u)
    nc = _get_nc()
    res = bass_utils.run_bass_kernel_spmd(
        nc,
        [dict(dev) for _ in range(N_CORES)],
        core_ids=list(range(N_CORES)),
        trace=_trace,
    )
    c_bits = np.asarray(res.results[0]["c_out"]).reshape(-1)
    if _trace:
        kernel.last_exec_time_ns = res.exec_time_ns
        kernel.last_results = res
    return assemble_output(c_bits, aux).astype(np.asarray(initial_state).dtype)
